# revision 23
# baseline (speedup 1.0000x reference)
"""MLA/GQA attention kernel v7 for Trainium2, 8-core SPMD.

Latent-space attention (W_k_from_latent absorbed into W_q, W_v_from_latent
+ W_o absorbed into an effective W_o), so S and PV contract over 64-dim
latents per head. 8 cores = 4 batches x 2 head-halves; each core does
8 heads (4 pairs j, pair = heads[j], heads[j+4] sharing kv slots 0/1).

Key structure (HW-measured wins vs the fp8-DoubleRow v3 baseline):
 - q/k latents in fp8e4 plain [128, T] layout (pair of heads / kv slots
   stacked on partitions): projection psum -> fp8 repack is one straight
   [128,1024] copy, and S matmuls are K=64 fp8 pairs at base partitions
   0 and 64 writing adjacent psum banks -- the two MMs run concurrently
   in different PE row-groups (~91ns/MM vs 443ns same-row-group).
 - S pairs are emitted one k-tile ahead of the PV matmuls so their
   LDWEIGHTS overlap the in-flight PVs.
 - one [128,1024] exp covers both heads; ~3/16 of exp tiles run on the
   DVE via a self-registered 1-pass polynomial op (EXP_POLY_ANT,
   exp(x) ~= p3(x/2)^2), the rest on ScalarE: both engines share the
   softmax-exp roof.
 - PV pairs accumulate into one [128,1024] ctx psum tile:
   bank0 = head A [ctx 0:64 | denom 64:128], bank1 = head B [denom|ctx]
   via the v-tile [g0 | ones | g1] ones-column trick.
 - 1/denominator via RECIPROCAL_APPROX_FAST (builtin reciprocal is
   ~6 cycles/elem on HW); custom DVE ops need base-partition-0 APs, so
   both heads' denominators are gathered into one base-0 tile first.
 - input DMAs split across the SP HWDGE queue and the Pool SWDGE queue.
"""
import sys

sys.path.insert(0, "/opt/trn_rl_repo")

import numpy as np
import ml_dtypes

import concourse.bass as bass  # noqa: F401
import concourse.mybir as mybir
import concourse.tile as tile
from concourse import bacc, bass_utils
from concourse import dve_ops as _dve_ops
from concourse.dve_spec import (
    Spec, Src0, C0, C1, C2, One, sq, lower, _has_src1 as has_src1,
)
from concourse.dve_uop import DveOpSpec


def _exp_poly_ref(in0, in1, s0, s1, imm2):
    u = in0.astype(np.float32) * s0
    p = (1.0 + u) + u * u * (imm2 + in0.astype(np.float32) * s1)
    return (p * p).astype(np.float32)


def _register_exp_poly():
    """exp(scale*x) ~= p3(scale*x/2)^2 via one 8-slice DVE pass.
    p3(u) = (1+u) + u^2*(1/2 + u/6); rel err ~ (scale*x)^4/192 -- fine for
    the |scale*S| <~ 0.7 regime here. s0=scale/2, s1=scale/12, imm2=0.5."""
    for op in _dve_ops.OPS:
        if op.name == "EXP_POLY_ANT":
            return op
    u = Src0 * C0
    body = sq((One + u) + (u * u) * (C2 + Src0 * C1))
    spec = Spec(body=body, reference=_exp_poly_ref)
    row = max(_dve_ops._SUB_OPCODE_FOR_NAME.values()) + 1
    assert row < 0x20
    _dve_ops._SUB_OPCODE_FOR_NAME["EXP_POLY_ANT"] = row
    shas = {}
    for ver in ("v3", "v4"):
        tmp = DveOpSpec(name="EXP_POLY_ANT", opcode=row,
                        uops=lower(spec, ver=ver), rd1_en=has_src1(spec))
        shas[ver] = tmp.sha(ver)
    op = _dve_ops.DveOp("EXP_POLY_ANT", spec, subdim=False, uops_sha=shas)
    _dve_ops.OPS.append(op)
    _dve_ops.CUSTOM_DVE_SPECS[op.name] = spec
    return op


EXP_POLY = _register_exp_poly()

import os as _os

_USE_DVEEXP = _os.environ.get("KV5_DVEEXP", "1") == "1"
_USE_RECFAST = _os.environ.get("KV5_RECFAST", "1") == "1"
_USE_GPDMA = _os.environ.get("KV5_GPDMA", "1") == "1"

D = 2048
T = 2048
NH = 16
NKV = 4
DH = 128
LAT = 64
B = 4
NCORE = 8
HQ = 8
NJ = 4
SCALE = 1.0 / np.sqrt(np.float32(DH))

NCC = D // 128
NT = T // 128
NQ = T // 512

F32 = mybir.dt.float32
BF16 = mybir.dt.bfloat16
FP8 = mybir.dt.float8e4
EXP = mybir.ActivationFunctionType.Exp

_CACHE = {}


def _build(reps=1, phases="all", prelude=2):
    prelude = int(prelude)  # number of query blocks handled in phase P
    nc = bacc.Bacc("TRN2", target_bir_lowering=False, debug=False)
    xt_d = nc.dram_tensor("xt", [D, T], BF16, kind="ExternalInput").ap()
    wq_d = nc.dram_tensor("wq", [D, NJ * 128], BF16, kind="ExternalInput").ap()
    wk_d = nc.dram_tensor("wk", [D, 128], BF16, kind="ExternalInput").ap()
    wv_d = nc.dram_tensor("wv", [D, 128], BF16, kind="ExternalInput").ap()
    wo_d = nc.dram_tensor("wo", [NJ * 128, D], BF16, kind="ExternalInput").ap()
    out_d = nc.dram_tensor("out", [T, D], BF16, kind="ExternalOutput").ap()

    with tile.TileContext(nc) as tc:
      for rep in range(reps):
        R = f"r{rep}"
        with tc.tile_pool(name=f"persist{R}", bufs=1) as persist:
            xts = [persist.tile([128, T], BF16, tag=f"x{c}{R}", name=f"x{c}{R}")
                   for c in range(NCC)]
            wqs = [persist.tile([128, NJ * 128], BF16, tag=f"wq{c}{R}",
                                name=f"wq{c}{R}") for c in range(NCC)]
            wks = [persist.tile([128, 128], BF16, tag=f"wk{c}{R}",
                                name=f"wk{c}{R}") for c in range(NCC)]
            wvs = [persist.tile([128, 128], BF16, tag=f"wv{c}{R}", name=f"wv{c}{R}")
                   for c in range(NCC)]
            wos = [persist.tile([128, D], BF16, tag=f"wo{j}{R}", name=f"wo{j}{R}")
                   for j in range(NJ)]
            # q8s[j]: [128, T] fp8 — partitions = [head j lat 64 | head j+4
            # lat 64]; kt same with kv slots 0/1
            q8s = [persist.tile([128, T], FP8, tag=f"q8{j}{R}", name=f"q8{j}{R}")
                   for j in range(NJ)]
            kt8 = persist.tile([128, T], FP8, tag=f"kt{R}", name=f"kt{R}")
            vts = [persist.tile([128, 192], BF16, tag=f"v{t}{R}", name=f"v{t}{R}")
                   for t in range(NT)]
            ctxp = [persist.tile([128, T], BF16, tag=f"c{j}{R}", name=f"c{j}{R}")
                    for j in range(NJ)]

            # xt + wk first (phase P head); spread across the SP HWDGE queue
            # and the Pool SWDGE queue so the load head is ~2x shorter
            for c in range(NCC):
                eng = nc.sync if (c % 2 == 0 or not _USE_GPDMA) else nc.gpsimd
                eng.dma_start(wks[c][:], wk_d[c * 128:(c + 1) * 128, :])
                eng.dma_start(xts[c][:], xt_d[c * 128:(c + 1) * 128, :])
            for c in range(NCC):
                eng = nc.sync if (c % 2 == 0 or not _USE_GPDMA) else nc.gpsimd
                eng.dma_start(wvs[c][:], wv_d[c * 128:(c + 1) * 128, :])
                eng.dma_start(wqs[c][:], wq_d[c * 128:(c + 1) * 128, :])
            for j in range(NJ):
                nc.sync.dma_start(wos[j][:], wo_d[j * 128:(j + 1) * 128, :])
            for t in range(NT):
                nc.vector.memset(vts[t][:], 1.0)

            with tc.tile_pool(name=f"actx{R}", bufs=1, space="PSUM") as acp, \
                 tc.tile_pool(name=f"aexp{R}", bufs=7) as aexp, \
                 tc.tile_pool(name=f"arec{R}", bufs=2) as arec, \
                 tc.tile_pool(name=f"aost{R}", bufs=2) as aost:

                def emit_head_pair(qc, j, spool):
                    """S + exp + PV for head pair j over query block qc.
                    S pairs are emitted one iteration ahead of the PVs so
                    their LDWEIGHTS overlap the previous PV matmuls."""
                    ps_ctx = acp.tile([128, 1024], F32, tag="ps_ctx",
                                      name=f"psc{qc}{j}{R}")
                    q0 = qc * 512

                    def s_pair(i):
                        ps_s = spool.tile([128, 1024], F32, tag="ps_s",
                                          name=f"pss{qc}{j}{i}{R}")
                        kcol = slice(i * 128, (i + 1) * 128)
                        nc.tensor.matmul(
                            ps_s[:, 0:512],
                            kt8[0:64, kcol],
                            q8s[j][0:64, q0:q0 + 512],
                            start=True, stop=True)
                        nc.tensor.matmul(
                            ps_s[:, 512:1024],
                            kt8[64:128, kcol],
                            q8s[j][64:128, q0:q0 + 512],
                            start=True, stop=True)
                        return ps_s

                    ps_cur = s_pair(0)
                    for i in range(NT):
                        ps_nxt = s_pair(i + 1) if i + 1 < NT else None
                        ex = aexp.tile([128, 1024], BF16, tag="expw",
                                       name=f"ex{qc}{j}{i}{R}")
                        if i % 5 == 4 and _USE_DVEEXP:
                            # offload ~3/16 of the exp tiles to the DVE
                            nc.vector._custom_dve(
                                EXP_POLY, out=ex[:], in0=ps_cur[:],
                                s0=float(SCALE) / 2,
                                s1=float(SCALE) / 12, imm2=0.5)
                        else:
                            nc.scalar.activation(ex[:], ps_cur[:], EXP,
                                                 scale=float(SCALE))
                        nc.tensor.matmul(
                            ps_ctx[:, 0:512],
                            vts[i][:, 0:128],
                            ex[:, 0:512],
                            start=(i == 0), stop=(i == NT - 1))
                        nc.tensor.matmul(
                            ps_ctx[:, 512:1024],
                            vts[i][:, 64:192],
                            ex[:, 512:1024],
                            start=(i == 0), stop=(i == NT - 1))
                        ps_cur = ps_nxt
                    rec = arec.tile([128, 512], F32, tag="rec",
                                    name=f"rec{qc}{j}{R}")
                    RC = _dve_ops.RECIP_APPROX_FAST_CONSTS

                    if _USE_RECFAST:
                        # custom ops need full-width base-0 APs: gather both
                        # heads' denominators into one [128,512] tile with
                        # builtin (shift-capable) copies, one recip for both
                        dn = arec.tile([128, 512], F32, tag="dn",
                                       name=f"dn{qc}{j}{R}")
                        nc.vector.tensor_copy(dn[0:64, :],
                                              ps_ctx[64:128, 0:512])
                        nc.vector.tensor_copy(dn[64:128, :],
                                              ps_ctx[0:64, 512:1024])
                        nc.vector._custom_dve(
                            _dve_ops.RECIPROCAL_APPROX_FAST,
                            out=rec[:, :], in0=dn[:, :], **RC)
                    else:
                        # head A denom: parts 64:128 of bank0; B: 0:64 of b1
                        nc.vector.reciprocal(rec[0:64, :],
                                             ps_ctx[64:128, 0:512])
                        nc.vector.reciprocal(rec[64:128, :],
                                             ps_ctx[0:64, 512:1024])
                    nc.vector.tensor_mul(
                        ctxp[j][0:64, q0:q0 + 512],
                        ps_ctx[0:64, 0:512], rec[0:64, :])
                    nc.vector.tensor_mul(
                        ctxp[j][64:128, q0:q0 + 512],
                        ps_ctx[64:128, 512:1024], rec[64:128, :])

                # ------------- Phase P (+ query-block-0 prelude) -----------
                pP_cm = tc.tile_pool(name=f"pP{R}", bufs=1, space="PSUM")
                pP = pP_cm.__enter__()
                sn_cm = tc.tile_pool(name=f"sn{R}", bufs=2, space="PSUM")
                sn = sn_cm.__enter__()
                # K proj -> kt8 fp8 (direct layout: [slot0 lat | slot1 lat])
                for half in range(2):
                    pk = pP.tile([128, 1024], F32, tag="pp",
                                 name=f"pk{half}{R}")
                    for c in range(NCC):
                        for f in range(2):
                            fo = half * 1024 + f * 512
                            nc.tensor.matmul(
                                pk[:, f * 512:(f + 1) * 512],
                                wks[c][:],
                                xts[c][:, fo:fo + 512],
                                start=(c == 0), stop=(c == NCC - 1))
                    nc.vector.tensor_copy(
                        kt8[:, half * 1024:(half + 1) * 1024], pk[:])
                # V proj (bf16, natural orientation)
                for r in range(NT // 2):
                    pv = pP.tile([128, 1024], F32, tag="pp", name=f"pv{r}{R}")
                    for c in range(NCC):
                        for tl in range(2):
                            tg = 2 * r + tl
                            nc.tensor.matmul(
                                pv[:, tl * 512:tl * 512 + 128],
                                xts[c][:, tg * 128:(tg + 1) * 128], wvs[c][:],
                                start=(c == 0), stop=(c == NCC - 1))
                    for tl in range(2):
                        tg = 2 * r + tl
                        nc.vector.tensor_copy(
                            vts[tg][:, 0:64], pv[:, tl * 512:tl * 512 + 64])
                        nc.vector.tensor_copy(
                            vts[tg][:, 128:192],
                            pv[:, tl * 512 + 64:tl * 512 + 128])
                # Q proj per pair j -> q8 fp8 direct, with qc0 prelude
                for j in range(NJ):
                    for half in range(2):
                        pq = pP.tile([128, 1024], F32, tag="pp",
                                     name=f"pq{j}{half}{R}")
                        for c in range(NCC):
                            for f in range(2):
                                fo = half * 1024 + f * 512
                                nc.tensor.matmul(
                                    pq[:, f * 512:(f + 1) * 512],
                                    wqs[c][:, j * 128:(j + 1) * 128],
                                    xts[c][:, fo:fo + 512],
                                    start=(c == 0), stop=(c == NCC - 1))
                        nc.vector.tensor_copy(
                            q8s[j][:, half * 1024:(half + 1) * 1024], pq[:])
                    for pq_ in range(prelude):
                        emit_head_pair(pq_, j, sn)

                if phases == "p":
                    for j in range(NJ):
                        nc.sync.dma_start(out_d[j * 128:(j + 1) * 128, :],
                                          ctxp[j][:] if prelude else xts[j][:])
                    sn_cm.__exit__(None, None, None)
                    pP_cm.__exit__(None, None, None)
                    continue

                sn_cm.__exit__(None, None, None)
                pP_cm.__exit__(None, None, None)

                # ---------------- Phase A+O ----------------
                with tc.tile_pool(name=f"as{R}", bufs=2, space="PSUM") as asp, \
                     tc.tile_pool(name=f"aoo{R}", bufs=2, space="PSUM") as aop:

                    pend = []
                    ostage = {}

                    def emit_o(tg, od):
                        if phases == "pa":
                            return
                        if od == 0:
                            ostage[tg] = aost.tile([128, D], BF16, tag="ost",
                                                   name=f"ost{tg}{R}")
                        oo = aop.tile([128, 512], F32, tag="oo",
                                      name=f"oo{tg}{od}{R}")
                        for j in range(NJ):
                            nc.tensor.matmul(
                                oo[:], ctxp[j][:, tg * 128:(tg + 1) * 128],
                                wos[j][:, od * 512:(od + 1) * 512],
                                start=(j == 0), stop=(j == NJ - 1))
                        st = ostage[tg]
                        nc.vector.tensor_copy(st[:, od * 512:(od + 1) * 512],
                                              oo[:])
                        if od == 3:
                            nc.sync.dma_start(
                                out_d[tg * 128:(tg + 1) * 128, :], st[:])
                            del ostage[tg]

                    qc0 = prelude
                    pend.extend((tg, od) for tg in range(4 * prelude)
                                for od in range(4))
                    npop = -(-16 * NQ // (4 * max(1, NQ - qc0)))
                    for qc in range(qc0, NQ):
                        for j in range(NJ):
                            emit_head_pair(qc, j, asp)
                            for _ in range(npop):
                                if pend:
                                    emit_o(*pend.pop(0))
                        pend.extend((tg, od)
                                    for tg in range(4 * qc, 4 * qc + 4)
                                    for od in range(4))
                    for g in pend:
                        emit_o(*g)
                    if phases == "pa":
                        for j in range(NJ):
                            nc.sync.dma_start(
                                out_d[j * 128:(j + 1) * 128, :], ctxp[j][:])

    nc.compile()
    return nc


LAST_RESULTS = None


def _prep_inputs(x, W_q, W_k, W_v, W_k_to_latent, W_v_to_latent,
                 W_k_from_latent, W_v_from_latent, W_o):
    x = np.asarray(x, np.float32)
    W_q = np.asarray(W_q, np.float32)
    W_k = np.asarray(W_k, np.float32)
    W_v = np.asarray(W_v, np.float32)
    W_ktl = np.asarray(W_k_to_latent, np.float32)
    W_vtl = np.asarray(W_v_to_latent, np.float32)
    W_kf = np.asarray(W_k_from_latent, np.float32)
    W_vf = np.asarray(W_v_from_latent, np.float32)
    W_o = np.asarray(W_o, np.float32)

    wq_eff = np.stack([W_q[:, h * DH:(h + 1) * DH] @ W_kf.T
                       for h in range(NH)], 1)          # [D, NH, LAT]
    wk_lat = np.stack([W_k[:, g * DH:(g + 1) * DH] @ W_ktl
                       for g in range(NKV)], 1)
    wv_lat = np.stack([W_v[:, g * DH:(g + 1) * DH] @ W_vtl
                       for g in range(NKV)], 1)
    wo_eff = np.stack([W_vf @ W_o[h * DH:(h + 1) * DH, :]
                       for h in range(NH)], 0)          # [NH, LAT, D]

    bf = ml_dtypes.bfloat16
    in_maps = []
    for c in range(NCORE):
        b, p = c // 2, c % 2
        heads = [8 * p + j for j in range(HQ)]
        wq_core = np.concatenate(
            [np.concatenate([wq_eff[:, heads[j]], wq_eff[:, heads[j + 4]]], 1)
             for j in range(NJ)], 1)
        wk_core = np.concatenate([wk_lat[:, 2 * p], wk_lat[:, 2 * p + 1]], 1)
        wv_core = np.concatenate([wv_lat[:, 2 * p], wv_lat[:, 2 * p + 1]], 1)
        wo_core = np.concatenate(
            [np.concatenate([wo_eff[heads[j]], wo_eff[heads[j + 4]]], 0)
             for j in range(NJ)], 0)
        xt = np.ascontiguousarray(x[b].T)
        in_maps.append({
            "xt": xt.astype(bf),
            "wq": np.ascontiguousarray(wq_core).astype(bf),
            "wk": np.ascontiguousarray(wk_core).astype(bf),
            "wv": np.ascontiguousarray(wv_core).astype(bf),
            "wo": np.ascontiguousarray(wo_core).astype(bf),
        })
    return in_maps


def kernel(x, W_q, W_k, W_v, W_k_to_latent, W_v_to_latent,
           W_k_from_latent, W_v_from_latent, W_o):
    global LAST_RESULTS
    in_maps = _prep_inputs(x, W_q, W_k, W_v, W_k_to_latent, W_v_to_latent,
                           W_k_from_latent, W_v_from_latent, W_o)
    if "nc" not in _CACHE:
        _CACHE["nc"] = _build()
    nc = _CACHE["nc"]
    res = bass_utils.run_bass_kernel_spmd(nc, in_maps, core_ids=list(range(NCORE)))
    LAST_RESULTS = res
    out = np.empty((B, T, D), np.float32)
    for b in range(B):
        out[b] = (res.results[2 * b]["out"].astype(np.float32)
                  + res.results[2 * b + 1]["out"].astype(np.float32))
    return out
